# revision 25
# baseline (speedup 1.0000x reference)
"""Distributed Trainium2 attention kernel (8 NeuronCores).

Strategy: tensor-parallel over heads for QKV projection + attention
(4 query heads + their 1 shared KV head per core, identical causal loop
structure on every core), then per-head AllToAlls switch to row-sharding
so each core computes the output projection for its 512 rows with the
full wo. Host reassembles rows. All matmuls run in bf16 with fp32 PSUM
accumulation; softmax runs unnormalized with the normalization folded in
after the PV matmul (per-head row sums via an all-ones-matmul whose
output is replicated across all 128 partitions, so no partition
broadcast is needed).

RoPE is applied in row-major layout via a host-side even/odd column
permutation of wq/wk (rotation becomes contiguous half-block arithmetic),
then q/k are transposed to [head_dim, rows] on the TensorEngine for the
attention matmuls.

Attention chains process key blocks in fused pairs: two score matmuls
land in one two-bank PSUM tile and a single wide Exp activation covers
both, halving the ScalarEngine instruction overhead.
"""

import numpy as np
import ml_dtypes

import concourse.bass as bass
import concourse.mybir as mybir
import concourse.tile as tile
from concourse import bacc
from concourse import bass_utils

B, S, D = 2, 2048, 4096
H, HKV, HD = 32, 8, 128
HD2 = HD // 2
NC = 8
HL = H // NC            # 4 local q heads per core
BS = B * S              # 4096 global rows
R = BS // NC            # 512 output rows per core
NRB = BS // 128         # 32 row blocks
NDT = D // 128          # 32 contraction tiles
SCALE = 1.0 / float(np.sqrt(HD))
BF = mybir.dt.bfloat16
F32 = mybir.dt.float32

PROFILE = False         # set by test.py for neuron-profile capture
TMPDIR = None           # set by test.py to keep the trace dir


def _emit(nc, tc, io):
    xT, wqkvT, woT, ccR, ssR, trim, ones, iden, out = io

    with (
        tc.tile_pool(name="ps1", bufs=4, space="PSUM") as ps1,
        tc.tile_pool(name="ps2", bufs=2, space="PSUM") as ps2,
        tc.tile_pool(name="wbuf", bufs=1) as wbuf,
        tc.tile_pool(name="qbuf", bufs=1) as qbuf,
        tc.tile_pool(name="kvbuf", bufs=1) as kvbuf,
        tc.tile_pool(name="abuf", bufs=1) as abuf,
        tc.tile_pool(name="cbuf", bufs=1) as cbuf,
        tc.tile_pool(name="xs", bufs=20) as xs,
        tc.tile_pool(name="cs", bufs=6) as cs,
        tc.tile_pool(name="es", bufs=6) as es,
        tc.tile_pool(name="ws", bufs=20) as ws,
        tc.tile_pool(name="ts", bufs=8) as ts,
        tc.tile_pool(name="ans", bufs=4) as ans,
        tc.tile_pool(name="rsp", bufs=3) as rsp,
        tc.tile_pool(name="os", bufs=3) as osp,
        tc.tile_pool(name="dram", bufs=1, space="DRAM") as dram,
    ):
        # ---- constants ----
        trim_sb = cbuf.tile([128, 128], F32, tag="tm")
        nc.sync.dma_start(trim_sb[:], trim[:])
        ones_sb = cbuf.tile([128, 128], BF, tag="on")
        nc.scalar.dma_start(ones_sb[:], ones[:])
        iden_sb = cbuf.tile([128, 128], BF, tag="idn")
        nc.gpsimd.dma_start(iden_sb[:], iden[:])

        # resident QKV weights: col = dt*768 + [0:512 q | 512:640 k | 640:768 v]
        # First half loaded up front (rotated across all three DMA queues so
        # the first row block is never starved); the rest is emitted inside
        # the rb loop to let rb0/rb1's x tiles through first.
        w_sb = wbuf.tile([128, NDT * 768], BF, tag="w")

        def w_load(dt):
            eng = (nc.sync, nc.scalar, nc.gpsimd)[dt % 3]
            eng.dma_start(
                w_sb[:, dt * 768: dt * 768 + 768],
                wqkvT[dt * 128: (dt + 1) * 128, :],
            )

        for dt in range(16):
            w_load(dt)

        q_sb = qbuf.tile([128, HL * BS], BF, tag="q")     # col = h*4096 + row
        kT_sb = kvbuf.tile([128, BS], BF, tag="k")        # col = row
        v_sb = kvbuf.tile([128, BS], BF, tag="v")         # col = rb*128 + hd

        # Four per-head AllToAlls: head h's chunk fires as soon as all 8 of
        # its chains drain, so only the last head's (small) collective is
        # exposed at the C->D boundary.
        a2a_in = [dram.tile([8 * 128, R], BF, name=f"a2a_in{h}") for h in range(HL)]
        a2a_out = [dram.tile([8 * 128, R], BF, name=f"a2a_out{h}") for h in range(HL)]

        # ---- phase B: QKV projection + RoPE + transposes ----
        # The rope+transpose tail of row block rb is emitted one iteration
        # late, behind rb+1's matmuls, so the PE queue never stalls on the
        # DVE rope chain.
        def b_rope_tail_q(rb, ps_q):
            cct = cs.tile([128, 256], BF, tag="cc")
            nc.sync.dma_start(cct[:], ccR[:, rb * 256: (rb + 1) * 256])
            sst = cs.tile([128, 256], BF, tag="ss")
            nc.sync.dma_start(sst[:], ssR[:, rb * 256: (rb + 1) * 256])

            # q rotation, all 4 heads at once via strided APs
            qe = ps_q.rearrange("p (h d) -> p h d", d=128)[:, :, 0:HD2]
            qo = ps_q.rearrange("p (h d) -> p h d", d=128)[:, :, HD2:HD]
            t1 = ts.tile([128, 256], BF, tag="t")
            t2 = ts.tile([128, 256], BF, tag="t")
            t3 = ts.tile([128, 256], BF, tag="t")
            t4 = ts.tile([128, 256], BF, tag="t")
            nc.vector.tensor_mul(t1[:], qe, cct[:])
            nc.vector.tensor_mul(t2[:], qo, sst[:])
            nc.vector.tensor_mul(t3[:], qe, sst[:])
            nc.vector.tensor_mul(t4[:], qo, cct[:])
            qrot = ts.tile([128, 512], BF, tag="qr")
            qre = qrot[:].rearrange("p (h d) -> p h d", d=128)[:, :, 0:HD2]
            qro = qrot[:].rearrange("p (h d) -> p h d", d=128)[:, :, HD2:HD]
            nc.vector.tensor_sub(qre, t1[:], t2[:])
            nc.vector.tensor_add(qro, t3[:], t4[:])
            return (qrot, cct, sst)

        def b_transpose_tail_q(rb, qrot):
            # transpose q (4 heads, packed into one psum bank pair)
            ps_tq = ps1.tile([128, 512], BF, tag="p1")
            for h in range(HL):
                nc.tensor.transpose(
                    ps_tq[:, h * 128: (h + 1) * 128],
                    qrot[:, h * 128: (h + 1) * 128],
                    iden_sb[:],
                )
            q_dst = (
                q_sb[:]
                .rearrange("p (h r) -> p h r", h=HL)
                [:, :, rb * 128: (rb + 1) * 128]
            )
            nc.vector.tensor_copy(
                q_dst, ps_tq[:].rearrange("p (h r) -> p h r", h=HL)
            )

        def b_rope_tail_kv(rb, ps_kv, cct, sst):
            # ps_kv: [128, 256] = [k | v]; k rotation (one head)
            ke = ps_kv[:, 0:HD2]
            ko = ps_kv[:, HD2:HD]
            u1 = ts.tile([128, 64], BF, tag="u")
            u2 = ts.tile([128, 64], BF, tag="u")
            u3 = ts.tile([128, 64], BF, tag="u")
            u4 = ts.tile([128, 64], BF, tag="u")
            nc.vector.tensor_mul(u1[:], ke, cct[:, 0:HD2])
            nc.vector.tensor_mul(u2[:], ko, sst[:, 0:HD2])
            nc.vector.tensor_mul(u3[:], ke, sst[:, 0:HD2])
            nc.vector.tensor_mul(u4[:], ko, cct[:, 0:HD2])
            krot = ts.tile([128, 128], BF, tag="kr")
            nc.vector.tensor_sub(krot[:, 0:HD2], u1[:], u2[:])
            nc.vector.tensor_add(krot[:, HD2:HD], u3[:], u4[:])

            # v: plain copy to row-major storage
            nc.scalar.activation(
                v_sb[:, rb * 128: (rb + 1) * 128], ps_kv[:, 128:256],
                mybir.ActivationFunctionType.Copy,
            )
            return (krot,)

        def b_transpose_tail_kv(rb, krot):
            ps_tk = ps1.tile([128, 128], BF, tag="p1")
            nc.tensor.transpose(ps_tk[:], krot[:], iden_sb[:])
            nc.vector.tensor_copy(kT_sb[:, rb * 128: (rb + 1) * 128], ps_tk[:])

        def b_block(rb, ps_q, ps_kv, dts, wload_base=None):
            for i, dt in enumerate(dts):
                if wload_base is not None and wload_base + i < NDT:
                    w_load(wload_base + i)
                xt = xs.tile([128, 128], BF, tag="x")
                eng = (nc.sync, nc.scalar, nc.gpsimd)[dt % 3]
                eng.dma_start(
                    xt[:], xT[dt * 128: (dt + 1) * 128, rb * 128: (rb + 1) * 128]
                )
                st, sp = dt == 0, dt == NDT - 1
                nc.tensor.matmul(
                    ps_q, xt[:], w_sb[:, dt * 768: dt * 768 + 512],
                    start=st, stop=sp,
                )
                nc.tensor.matmul(
                    ps_kv, xt[:], w_sb[:, dt * 768 + 512: dt * 768 + 768],
                    start=st, stop=sp,
                )

        # rb0/rb1 run their first contraction halves back to back so the
        # second half of the resident weights (w16..31, 3 MB) has a full
        # extra row-block of matmul time to stream in: the startup is DMA
        # bandwidth bound, not PE bound.
        ps_qkv0 = ps2.tile([128, 1024], F32, tag="p2")
        q0p, kv0p = ps_qkv0[:, 0:512], ps_qkv0[:, 512:768]
        ps_qkv1 = ps2.tile([128, 1024], F32, tag="p2")
        q1p, kv1p = ps_qkv1[:, 0:512], ps_qkv1[:, 512:768]
        b_block(0, q0p, kv0p, range(16))
        b_block(1, q1p, kv1p, range(16), wload_base=16)
        b_block(0, q0p, kv0p, range(16, 32))
        qr0, cct0, sst0 = b_rope_tail_q(0, q0p)
        rot0 = (0, qr0) + b_rope_tail_kv(0, kv0p, cct0, sst0)
        b_block(1, q1p, kv1p, range(16, 32))
        b_transpose_tail_q(rot0[0], rot0[1])
        b_transpose_tail_kv(rot0[0], rot0[2])

        pending = (1, q1p, kv1p)
        rot = None
        for rb in range(2, NRB):
            # [0:512] = 4 q heads, [512:640] = k, [640:768] = v
            ps_qkv = ps2.tile([128, 1024], F32, tag="p2")
            ps_q = ps_qkv[:, 0:512]
            ps_kv = ps_qkv[:, 512:768]
            for dt in range(NDT):
                xt = xs.tile([128, 128], BF, tag="x")
                eng = (nc.sync, nc.scalar, nc.gpsimd)[dt % 3]
                eng.dma_start(
                    xt[:], xT[dt * 128: (dt + 1) * 128, rb * 128: (rb + 1) * 128]
                )
                st, sp = dt == 0, dt == NDT - 1
                nc.tensor.matmul(
                    ps_q, xt[:], w_sb[:, dt * 768: dt * 768 + 512],
                    start=st, stop=sp,
                )
                nc.tensor.matmul(
                    ps_kv, xt[:], w_sb[:, dt * 768 + 512: dt * 768 + 768],
                    start=st, stop=sp,
                )
                if dt == 2 and pending is not None:
                    prb, pq, pkv = pending
                    qr, cct, sst = b_rope_tail_q(prb, pq)
                    rot = (prb, qr) + b_rope_tail_kv(prb, pkv, cct, sst)
                    pending = None
                if dt == 12 and rot is not None:
                    b_transpose_tail_q(rot[0], rot[1])
                    b_transpose_tail_kv(rot[0], rot[2])
                    rot = None
            pending = (rb, ps_q, ps_kv)
        prb, pq, pkv = pending
        qr, cct, sst = b_rope_tail_q(prb, pq)
        rot = (prb, qr) + b_rope_tail_kv(prb, pkv, cct, sst)
        b_transpose_tail_q(rot[0], rot[1])
        b_transpose_tail_kv(rot[0], rot[2])

        # ---- phase C: causal attention, paired interleaved chains ----
        # Each (b, h, ci) is an independent chain covering query rows
        # [512*ci, 512*ci+512) of batch b; key blocks are processed in
        # fused pairs (one 2-bank PSUM tile, one wide Exp).  Two chains are
        # emitted interleaved so one chain's exp latency hides under the
        # other's matmuls.
        def attn_chain(b, h, ci):
            qbase = h * BS + b * S
            pa = ps1.tile([128, 512], F32, tag="p1", name=f"pa{b}{h}{ci}")
            pr = ps1.tile([128, 512], F32, tag="p1", name=f"pr{b}{h}{ci}")
            jmax = 4 * ci + 3

            def qspan(j):
                q0 = max(j * 128, 512 * ci)
                return q0, 512 * ci + 512 - q0

            js = list(range(jmax + 1))
            groups = [js[i:i + 2] for i in range(0, len(js), 2)]

            def scores_pair(g):
                ps_s = ps2.tile([128, 1024], F32, tag="p2", name=f"s{b}{h}{ci}")
                offs = []
                off = 0
                for j in g:
                    q0, w = qspan(j)
                    kcol = (b * 16 + j) * 128
                    nc.tensor.matmul(
                        ps_s[:, off: off + w],
                        kT_sb[:, kcol: kcol + 128],
                        q_sb[:, qbase + q0: qbase + q0 + w],
                        start=True, stop=True,
                    )
                    if j // 4 == ci:
                        nc.vector.tensor_add(
                            ps_s[:, off: off + 128], ps_s[:, off: off + 128],
                            trim_sb[:],
                        )
                    offs.append((j, off, w))
                    # second matmul goes to the next bank boundary when the
                    # first filled its bank, else packs right behind it
                    off = 512 if w == 512 else off + w
                total = offs[-1][1] + offs[-1][2]
                et = es.tile([128, 1024], BF, tag="e", name=f"e{b}{h}{ci}")
                nc.scalar.activation(
                    et[:, 0:total], ps_s[:, 0:total],
                    mybir.ActivationFunctionType.Exp, scale=SCALE,
                )
                return et, offs

            def pv_pair(et, offs):
                # rowsums first: the chain-tail reciprocal then overlaps
                # the trailing PV matmuls instead of serializing after them
                for j, off, w in offs:
                    loc = max(j * 128, 512 * ci) - 512 * ci
                    nc.tensor.matmul(
                        pr[:, loc: loc + w],
                        ones_sb[:],
                        et[:, off: off + w],
                        start=(j == 0), stop=(j == jmax),
                    )
                for j, off, w in offs:
                    kcol = (b * 16 + j) * 128
                    loc = max(j * 128, 512 * ci) - 512 * ci
                    nc.tensor.matmul(
                        pa[:, loc: loc + w],
                        v_sb[:, kcol: kcol + 128],
                        et[:, off: off + w],
                        start=(j == 0), stop=(j == jmax),
                    )

            prev = None
            for g in groups:
                cur = scores_pair(g)
                if prev is not None:
                    pv_pair(*prev)
                prev = cur
                yield
            pv_pair(*prev)
            rc = rsp.tile([128, 512], F32, tag="rc")
            nc.vector.reciprocal_approx_fast(rc[:], pr[:])
            an = ans.tile([128, 512], BF, tag="an")
            nc.vector.tensor_mul(an[:], pa[:], rc[:])
            blk = 128 * (b * 4 + ci)
            nc.sync.dma_start(a2a_in[h][blk: blk + 128, :], an[:])
            yield

        # Continuous worklist in head-major order; after a head's 8 chains
        # all drain, its AllToAll fires while later heads keep computing.
        # per-head order staggers chain lengths across the two lanes so
        # two chains rarely drain at the same time (a simultaneous drain
        # idles the PE for ~1.5us)
        todo = []
        for h in range(HL):
            for b, ci in ((0, 3), (0, 2), (0, 1), (1, 3), (1, 2),
                          (0, 0), (1, 1), (1, 0)):
                todo.append((b, h, ci))
        todo.reverse()

        # at_sb: col = ht*512 + row; tiles stream in per-head as each
        # AllToAll completes.  The first 8 wo tiles for cg 0 are prefetched
        # mid-attention so phase D's accumulation starts immediately.
        at_sb = abuf.tile([128, 32 * 512], BF, tag="at")
        ht_order = [4 * i + l for l in range(HL) for i in range(8)]
        pre_wt = {}

        def at_load(eng, ht):
            i, htl = ht // 4, ht % 4
            eng.dma_start(
                at_sb[:, ht * 512: (ht + 1) * 512],
                a2a_out[htl][i * 128: (i + 1) * 128, :],
            )

        done_per_head = [0] * HL
        fired = 0

        def note_done(h):
            nonlocal fired
            done_per_head[h] += 1
            while fired < HL and done_per_head[fired] == 2 * 4:
                nc.gpsimd.collective_compute(
                    "AllToAll",
                    mybir.AluOpType.bypass,
                    replica_groups=[list(range(NC))],
                    ins=[a2a_in[fired].opt()],
                    outs=[a2a_out[fired].opt()],
                )
                if fired == 0:
                    for ht in ht_order[:8]:
                        wt = ws.tile([128, 512], BF, tag="wo")
                        nc.sync.dma_start(
                            wt[:], woT[ht * 128: (ht + 1) * 128, 0:512]
                        )
                        pre_wt[ht] = wt
                else:
                    # at loads for head h sit on the gpsimd queue right
                    # after collective h+1's trigger: their data (collective
                    # h) is already delivered, so they dispatch immediately
                    # and never block a queue that has pending work.
                    for ht in ht_order[(fired - 1) * 8: fired * 8]:
                        at_load(nc.gpsimd, ht)
                fired += 1

        def start(spec):
            return (spec[1], attn_chain(*spec))

        active = [start(todo.pop()), start(todo.pop())]
        while active:
            for item in list(active):
                h, g = item
                if next(g, StopIteration) is StopIteration:
                    active.remove(item)
                    note_done(h)
                    if todo:
                        active.append(start(todo.pop()))
        for ht in ht_order[(HL - 1) * 8:]:
            at_load(nc.gpsimd, ht)

        # ---- phase D: output projection for this core's 512 rows ----
        # ht order follows AllToAll arrival: all tiles of local head 0
        # first, then head 1, etc., so accumulation overlaps the later
        # collectives.  Remaining at_sb loads are emitted just before the
        # first wo tile that consumes them, on the opposite DMA queue.
        def d_alloc(cg):
            # alternate accumulator pools so cg+1's matmuls start while
            # cg's banks are still draining through the output copies
            if cg % 2 == 0:
                return [
                    ps1.tile([128, 512], F32, tag="p1", name=f"po{cg}_{i}")[:]
                    for i in range(4)
                ]
            pp = [
                ps2.tile([128, 1024], F32, tag="p2", name=f"pp{cg}_{i}")
                for i in range(2)
            ]
            return [pp[0][:, 0:512], pp[0][:, 512:1024],
                    pp[1][:, 0:512], pp[1][:, 512:1024]]

        def d_accum(cg, po, lo, hi):
            for n_ht in range(lo, hi):
                ht = ht_order[n_ht]
                if cg == 0 and ht in pre_wt:
                    wt = pre_wt[ht]
                else:
                    wt = ws.tile([128, 512], BF, tag="wo")
                    weng = (nc.sync, nc.scalar)[n_ht % 2]
                    weng.dma_start(
                        wt[:],
                        woT[ht * 128: (ht + 1) * 128, cg * 512: (cg + 1) * 512],
                    )
                for rt in range(4):
                    nc.tensor.matmul(
                        po[rt],
                        at_sb[:, ht * 512 + rt * 128: ht * 512 + (rt + 1) * 128],
                        wt[:],
                        start=(n_ht == 0), stop=(n_ht == 31),
                    )

        def d_drain(cg, po):
            for rt in range(4):
                ot = osp.tile([128, 512], F32, tag="o")
                nc.vector.tensor_copy(ot[:], po[rt])
                nc.scalar.dma_start(
                    out[rt * 128: (rt + 1) * 128, cg * 512: (cg + 1) * 512], ot[:]
                )

        # cg0/cg1 run as two interleaved passes: cg0's tail (which needs
        # the last AllToAll's tiles) is emitted after cg1's independent
        # head, so the PE queue never head-of-line blocks on collective 3.
        po0, po1 = d_alloc(0), d_alloc(1)
        d_accum(0, po0, 0, 20)
        d_accum(1, po1, 0, 20)
        d_accum(0, po0, 20, 32)
        d_drain(0, po0)
        d_accum(1, po1, 20, 32)
        d_drain(1, po1)
        for cg in range(2, 8):
            po = d_alloc(cg)
            d_accum(cg, po, 0, 32)
            d_drain(cg, po)


def _build():
    nc = bacc.Bacc("TRN2", target_bir_lowering=False, debug=False, num_devices=NC)
    xT = nc.dram_tensor("xT", [D, BS], BF, kind="ExternalInput")
    wqkvT = nc.dram_tensor("wqkvT", [D, 768], BF, kind="ExternalInput")
    woT = nc.dram_tensor("woT", [D, D], BF, kind="ExternalInput")
    ccR = nc.dram_tensor("ccR", [128, NRB * 256], BF, kind="ExternalInput")
    ssR = nc.dram_tensor("ssR", [128, NRB * 256], BF, kind="ExternalInput")
    trim = nc.dram_tensor("trim", [128, 128], F32, kind="ExternalInput")
    ones = nc.dram_tensor("ones", [128, 128], BF, kind="ExternalInput")
    iden = nc.dram_tensor("iden", [128, 128], BF, kind="ExternalInput")
    out = nc.dram_tensor("out", [R, D], F32, kind="ExternalOutput")
    with tile.TileContext(nc) as tc:
        _emit(nc, tc, (xT, wqkvT, woT, ccR, ssR, trim, ones, iden, out))
    nc.compile()
    return nc


_NC = None


def kernel(x, wq, wk, wv, wo, freqs_cos, freqs_sin, mask, start_pos):
    global _NC
    if _NC is None:
        _NC = _build()
    nc = _NC
    bf = ml_dtypes.bfloat16

    x = np.asarray(x, dtype=np.float32)
    xT = np.ascontiguousarray(x.reshape(BS, D).T).astype(bf)

    perm = np.concatenate([np.arange(0, HD, 2), np.arange(1, HD, 2)])
    wqTp = np.asarray(wq, np.float32).T.reshape(D, H, HD)[:, :, perm]
    wkTp = np.asarray(wk, np.float32).T.reshape(D, HKV, HD)[:, :, perm]
    wvT = np.asarray(wv, np.float32).T.reshape(D, HKV, HD)
    woT = np.ascontiguousarray(np.asarray(wo, np.float32).T).astype(bf)

    fc = np.asarray(freqs_cos, np.float32)
    fs = np.asarray(freqs_sin, np.float32)
    # row-major RoPE tables per row block, replicated x4 along free axis
    pos = (np.arange(BS) % S).reshape(NRB, 128)
    ccR = np.tile(fc[pos], (1, 1, 4)).transpose(1, 0, 2).reshape(128, NRB * 256)
    ssR = np.tile(fs[pos], (1, 1, 4)).transpose(1, 0, 2).reshape(128, NRB * 256)
    ccR = np.ascontiguousarray(ccR).astype(bf)
    ssR = np.ascontiguousarray(ssR).astype(bf)

    trim = np.where(
        np.arange(128)[:, None] > np.arange(128)[None, :], -1e30, 0.0
    ).astype(np.float32)
    ones = np.ones((128, 128), dtype=bf)
    iden = np.eye(128, dtype=bf)

    in_maps = []
    for c in range(NC):
        wqkv = np.concatenate(
            [
                wqTp[:, 4 * c: 4 * c + 4].reshape(D, 512),
                wkTp[:, c],
                wvT[:, c],
            ],
            axis=1,
        ).astype(bf)
        in_maps.append(
            {
                "xT": xT,
                "wqkvT": np.ascontiguousarray(wqkv),
                "woT": woT,
                "ccR": ccR,
                "ssR": ssR,
                "trim": trim,
                "ones": ones,
                "iden": iden,
            }
        )

    res = bass_utils.run_bass_kernel_spmd(
        nc, in_maps, core_ids=list(range(NC)), trace=PROFILE, tmpdir=TMPDIR
    )
    if PROFILE:
        print(f"HW exec time: {res.exec_time_ns} ns")
        if res.instructions_and_trace is not None:
            print(f"trace: {res.instructions_and_trace[1]}")

    out_full = np.empty((BS, D), dtype=np.float32)
    for c in range(NC):
        out_full[R * c: R * (c + 1)] = res.results[c]["out"]
    return out_full.reshape(B, S, D)


# revision 28
# speedup vs baseline: 1.0349x; 1.0349x over previous
"""Distributed Trainium2 attention kernel (8 NeuronCores).

Strategy: tensor-parallel over heads for QKV projection + attention
(4 query heads + their 1 shared KV head per core, identical causal loop
structure on every core), then per-head AllToAlls switch to row-sharding
so each core computes the output projection for its 512 rows with the
full wo. Host reassembles rows. All matmuls run in bf16 with fp32 PSUM
accumulation; softmax runs unnormalized with the normalization folded in
after the PV matmul (per-head row sums via an all-ones-matmul whose
output is replicated across all 128 partitions, so no partition
broadcast is needed).

RoPE is applied in row-major layout via a host-side even/odd column
permutation of wq/wk (rotation becomes contiguous half-block arithmetic),
then q/k are transposed to [head_dim, rows] on the TensorEngine for the
attention matmuls.

Attention chains process key blocks in fused pairs: two score matmuls
land in one two-bank PSUM tile and a single wide Exp activation covers
both, halving the ScalarEngine instruction overhead.
"""

import numpy as np
import ml_dtypes

import concourse.bass as bass
import concourse.mybir as mybir
import concourse.tile as tile
from concourse import bacc
from concourse import bass_utils

B, S, D = 2, 2048, 4096
H, HKV, HD = 32, 8, 128
HD2 = HD // 2
NC = 8
HL = H // NC            # 4 local q heads per core
BS = B * S              # 4096 global rows
R = BS // NC            # 512 output rows per core
NRB = BS // 128         # 32 row blocks
NDT = D // 128          # 32 contraction tiles
SCALE = 1.0 / float(np.sqrt(HD))
BF = mybir.dt.bfloat16
F32 = mybir.dt.float32

PROFILE = False         # set by test.py for neuron-profile capture
TMPDIR = None           # set by test.py to keep the trace dir


def _emit(nc, tc, io):
    xT, wqkvT, woT, ccR, ssR, trim, ones, iden, out = io

    with (
        tc.tile_pool(name="ps1", bufs=4, space="PSUM") as ps1,
        tc.tile_pool(name="ps2", bufs=2, space="PSUM") as ps2,
        tc.tile_pool(name="wbuf", bufs=1) as wbuf,
        tc.tile_pool(name="qbuf", bufs=1) as qbuf,
        tc.tile_pool(name="kvbuf", bufs=1) as kvbuf,
        tc.tile_pool(name="abuf", bufs=1) as abuf,
        tc.tile_pool(name="cbuf", bufs=1) as cbuf,
        tc.tile_pool(name="xs", bufs=20) as xs,
        tc.tile_pool(name="cs", bufs=6) as cs,
        tc.tile_pool(name="es", bufs=6) as es,
        tc.tile_pool(name="ws", bufs=20) as ws,
        tc.tile_pool(name="ts", bufs=8) as ts,
        tc.tile_pool(name="ans", bufs=4) as ans,
        tc.tile_pool(name="rsp", bufs=3) as rsp,
        tc.tile_pool(name="os", bufs=3) as osp,
        tc.tile_pool(name="dram", bufs=1, space="DRAM") as dram,
    ):
        # ---- constants ----
        trim_sb = cbuf.tile([128, 128], F32, tag="tm")
        nc.sync.dma_start(trim_sb[:], trim[:])
        ones_sb = cbuf.tile([128, 128], BF, tag="on")
        nc.scalar.dma_start(ones_sb[:], ones[:])
        iden_sb = cbuf.tile([128, 128], BF, tag="idn")
        nc.gpsimd.dma_start(iden_sb[:], iden[:])

        # resident QKV weights: col = dt*768 + [0:512 q | 512:640 k | 640:768 v]
        # First half loaded up front (rotated across all three DMA queues so
        # the first row block is never starved); the rest is emitted inside
        # the rb loop to let rb0/rb1's x tiles through first.
        w_sb = wbuf.tile([128, NDT * 768], BF, tag="w")

        def w_load(dt):
            eng = (nc.sync, nc.scalar, nc.gpsimd)[dt % 3]
            eng.dma_start(
                w_sb[:, dt * 768: dt * 768 + 768],
                wqkvT[dt * 128: (dt + 1) * 128, :],
            )

        for dt in range(16):
            w_load(dt)

        q_sb = qbuf.tile([128, HL * BS], BF, tag="q")     # col = h*4096 + row
        kT_sb = kvbuf.tile([128, BS], BF, tag="k")        # col = row
        v_sb = kvbuf.tile([128, BS], BF, tag="v")         # col = rb*128 + hd

        # Four per-head AllToAlls: head h's chunk fires as soon as all 8 of
        # its chains drain, so only the last head's (small) collective is
        # exposed at the C->D boundary.
        a2a_in = [dram.tile([8 * 128, R], BF, name=f"a2a_in{h}") for h in range(HL)]
        a2a_out = [dram.tile([8 * 128, R], BF, name=f"a2a_out{h}") for h in range(HL)]

        # ---- phase B: QKV projection + RoPE + transposes ----
        # The rope+transpose tail of row block rb is emitted one iteration
        # late, behind rb+1's matmuls, so the PE queue never stalls on the
        # DVE rope chain.
        def b_rope_tail_q(rb, ps_q):
            cct = cs.tile([128, 256], BF, tag="cc")
            nc.sync.dma_start(cct[:], ccR[:, rb * 256: (rb + 1) * 256])
            sst = cs.tile([128, 256], BF, tag="ss")
            nc.sync.dma_start(sst[:], ssR[:, rb * 256: (rb + 1) * 256])

            # q rotation, all 4 heads at once via strided APs
            qe = ps_q.rearrange("p (h d) -> p h d", d=128)[:, :, 0:HD2]
            qo = ps_q.rearrange("p (h d) -> p h d", d=128)[:, :, HD2:HD]
            t1 = ts.tile([128, 256], BF, tag="t")
            t2 = ts.tile([128, 256], BF, tag="t")
            t3 = ts.tile([128, 256], BF, tag="t")
            t4 = ts.tile([128, 256], BF, tag="t")
            nc.vector.tensor_mul(t1[:], qe, cct[:])
            nc.vector.tensor_mul(t2[:], qo, sst[:])
            nc.vector.tensor_mul(t3[:], qe, sst[:])
            nc.vector.tensor_mul(t4[:], qo, cct[:])
            qrot = ts.tile([128, 512], BF, tag="qr")
            qre = qrot[:].rearrange("p (h d) -> p h d", d=128)[:, :, 0:HD2]
            qro = qrot[:].rearrange("p (h d) -> p h d", d=128)[:, :, HD2:HD]
            nc.vector.tensor_sub(qre, t1[:], t2[:])
            nc.vector.tensor_add(qro, t3[:], t4[:])
            return (qrot, cct, sst)

        def b_transpose_tail_q(rb, qrot):
            # transpose q (4 heads, packed into one psum bank pair)
            ps_tq = ps1.tile([128, 512], BF, tag="p1")
            for h in range(HL):
                nc.tensor.transpose(
                    ps_tq[:, h * 128: (h + 1) * 128],
                    qrot[:, h * 128: (h + 1) * 128],
                    iden_sb[:],
                )
            q_dst = (
                q_sb[:]
                .rearrange("p (h r) -> p h r", h=HL)
                [:, :, rb * 128: (rb + 1) * 128]
            )
            nc.vector.tensor_copy(
                q_dst, ps_tq[:].rearrange("p (h r) -> p h r", h=HL)
            )

        def b_rope_tail_kv(rb, ps_kv, cct, sst):
            # ps_kv: [128, 256] = [k | v]; k rotation (one head)
            ke = ps_kv[:, 0:HD2]
            ko = ps_kv[:, HD2:HD]
            u1 = ts.tile([128, 64], BF, tag="u")
            u2 = ts.tile([128, 64], BF, tag="u")
            u3 = ts.tile([128, 64], BF, tag="u")
            u4 = ts.tile([128, 64], BF, tag="u")
            nc.vector.tensor_mul(u1[:], ke, cct[:, 0:HD2])
            nc.vector.tensor_mul(u2[:], ko, sst[:, 0:HD2])
            nc.vector.tensor_mul(u3[:], ke, sst[:, 0:HD2])
            nc.vector.tensor_mul(u4[:], ko, cct[:, 0:HD2])
            krot = ts.tile([128, 128], BF, tag="kr")
            nc.vector.tensor_sub(krot[:, 0:HD2], u1[:], u2[:])
            nc.vector.tensor_add(krot[:, HD2:HD], u3[:], u4[:])

            # v: plain copy to row-major storage
            nc.scalar.activation(
                v_sb[:, rb * 128: (rb + 1) * 128], ps_kv[:, 128:256],
                mybir.ActivationFunctionType.Copy,
            )
            return (krot,)

        def b_transpose_tail_kv(rb, krot):
            ps_tk = ps1.tile([128, 128], BF, tag="p1")
            nc.tensor.transpose(ps_tk[:], krot[:], iden_sb[:])
            nc.vector.tensor_copy(kT_sb[:, rb * 128: (rb + 1) * 128], ps_tk[:])

        pending = None
        rot = None
        for rb in range(NRB):
            # [0:512] = 4 q heads, [512:640] = k, [640:768] = v
            ps_qkv = ps2.tile([128, 1024], F32, tag="p2")
            ps_q = ps_qkv[:, 0:512]
            ps_kv = ps_qkv[:, 512:768]
            for dt in range(NDT):
                if rb == 0 and 14 <= dt < 30:
                    w_load(dt + 2)
                xt = xs.tile([128, 128], BF, tag="x")
                eng = (nc.sync, nc.scalar, nc.gpsimd)[dt % 3]
                eng.dma_start(
                    xt[:], xT[dt * 128: (dt + 1) * 128, rb * 128: (rb + 1) * 128]
                )
                st, sp = dt == 0, dt == NDT - 1
                nc.tensor.matmul(
                    ps_q, xt[:], w_sb[:, dt * 768: dt * 768 + 512],
                    start=st, stop=sp,
                )
                nc.tensor.matmul(
                    ps_kv, xt[:], w_sb[:, dt * 768 + 512: dt * 768 + 768],
                    start=st, stop=sp,
                )
                if dt == 2 and pending is not None:
                    prb, pq, pkv = pending
                    qr, cct, sst = b_rope_tail_q(prb, pq)
                    rot = (prb, qr) + b_rope_tail_kv(prb, pkv, cct, sst)
                    pending = None
                if dt == 12 and rot is not None:
                    b_transpose_tail_q(rot[0], rot[1])
                    b_transpose_tail_kv(rot[0], rot[2])
                    rot = None
            pending = (rb, ps_q, ps_kv)
        prb, pq, pkv = pending
        qr, cct, sst = b_rope_tail_q(prb, pq)
        rot = (prb, qr) + b_rope_tail_kv(prb, pkv, cct, sst)
        b_transpose_tail_q(rot[0], rot[1])
        b_transpose_tail_kv(rot[0], rot[2])

        # ---- phase C: causal attention, paired interleaved chains ----
        # Each (b, h, ci) is an independent chain covering query rows
        # [512*ci, 512*ci+512) of batch b; key blocks are processed in
        # fused pairs (one 2-bank PSUM tile, one wide Exp).  Two chains are
        # emitted interleaved so one chain's exp latency hides under the
        # other's matmuls.
        def attn_chain(b, h, ci):
            qbase = h * BS + b * S
            pa = ps1.tile([128, 512], F32, tag="p1", name=f"pa{b}{h}{ci}")
            pr = ps1.tile([128, 512], F32, tag="p1", name=f"pr{b}{h}{ci}")
            jmax = 4 * ci + 3

            def qspan(j):
                q0 = max(j * 128, 512 * ci)
                return q0, 512 * ci + 512 - q0

            js = list(range(jmax + 1))
            groups = [js[i:i + 2] for i in range(0, len(js), 2)]

            def scores_pair(g):
                ps_s = ps2.tile([128, 1024], F32, tag="p2", name=f"s{b}{h}{ci}")
                offs = []
                off = 0
                for j in g:
                    q0, w = qspan(j)
                    kcol = (b * 16 + j) * 128
                    nc.tensor.matmul(
                        ps_s[:, off: off + w],
                        kT_sb[:, kcol: kcol + 128],
                        q_sb[:, qbase + q0: qbase + q0 + w],
                        start=True, stop=True,
                    )
                    if j // 4 == ci:
                        nc.vector.tensor_add(
                            ps_s[:, off: off + 128], ps_s[:, off: off + 128],
                            trim_sb[:],
                        )
                    offs.append((j, off, w))
                    # second matmul goes to the next bank boundary when the
                    # first filled its bank, else packs right behind it
                    off = 512 if w == 512 else off + w
                total = offs[-1][1] + offs[-1][2]
                et = es.tile([128, 1024], BF, tag="e", name=f"e{b}{h}{ci}")
                nc.scalar.activation(
                    et[:, 0:total], ps_s[:, 0:total],
                    mybir.ActivationFunctionType.Exp, scale=SCALE,
                )
                return et, offs

            def pv_pair(et, offs):
                # rowsums first: the chain-tail reciprocal then overlaps
                # the trailing PV matmuls instead of serializing after them
                for j, off, w in offs:
                    loc = max(j * 128, 512 * ci) - 512 * ci
                    nc.tensor.matmul(
                        pr[:, loc: loc + w],
                        ones_sb[:],
                        et[:, off: off + w],
                        start=(j == 0), stop=(j == jmax),
                    )
                for j, off, w in offs:
                    kcol = (b * 16 + j) * 128
                    loc = max(j * 128, 512 * ci) - 512 * ci
                    nc.tensor.matmul(
                        pa[:, loc: loc + w],
                        v_sb[:, kcol: kcol + 128],
                        et[:, off: off + w],
                        start=(j == 0), stop=(j == jmax),
                    )

            prev = None
            for g in groups:
                cur = scores_pair(g)
                if prev is not None:
                    pv_pair(*prev)
                prev = cur
                yield
            pv_pair(*prev)
            rc = rsp.tile([128, 512], F32, tag="rc")
            nc.vector.reciprocal_approx_fast(rc[:], pr[:])
            an = ans.tile([128, 512], BF, tag="an")
            nc.vector.tensor_mul(an[:], pa[:], rc[:])
            blk = 128 * (b * 4 + ci)
            nc.sync.dma_start(a2a_in[h][blk: blk + 128, :], an[:])
            yield

        # Continuous worklist in head-major order; after a head's 8 chains
        # all drain, its AllToAll fires while later heads keep computing.
        todo = []
        for h in range(HL):
            for ci in (3, 2, 1, 0):
                for b in range(B):
                    todo.append((b, h, ci))
        todo.reverse()

        # at_sb: col = ht*512 + row; tiles stream in per-head as each
        # AllToAll completes.  The first 8 wo tiles for cg 0 are prefetched
        # mid-attention so phase D's accumulation starts immediately.
        at_sb = abuf.tile([128, 32 * 512], BF, tag="at")
        ht_order = [4 * i + l for l in range(HL) for i in range(8)]
        pre_wt = {}

        def at_load(eng, ht):
            i, htl = ht // 4, ht % 4
            eng.dma_start(
                at_sb[:, ht * 512: (ht + 1) * 512],
                a2a_out[htl][i * 128: (i + 1) * 128, :],
            )

        done_per_head = [0] * HL
        fired = 0

        def note_done(h):
            nonlocal fired
            done_per_head[h] += 1
            while fired < HL and done_per_head[fired] == 2 * 4:
                nc.gpsimd.collective_compute(
                    "AllToAll",
                    mybir.AluOpType.bypass,
                    replica_groups=[list(range(NC))],
                    ins=[a2a_in[fired].opt()],
                    outs=[a2a_out[fired].opt()],
                )
                if fired == 0:
                    for ht in ht_order[:8]:
                        wt = ws.tile([128, 512], BF, tag="wo")
                        nc.sync.dma_start(
                            wt[:], woT[ht * 128: (ht + 1) * 128, 0:512]
                        )
                        pre_wt[ht] = wt
                else:
                    # at loads for head h sit on the gpsimd queue right
                    # after collective h+1's trigger: their data (collective
                    # h) is already delivered, so they dispatch immediately
                    # and never block a queue that has pending work.
                    for ht in ht_order[(fired - 1) * 8: fired * 8]:
                        at_load(nc.gpsimd, ht)
                fired += 1

        def start(spec):
            return (spec[1], attn_chain(*spec))

        active = [start(todo.pop()), start(todo.pop())]
        while active:
            for item in list(active):
                h, g = item
                if next(g, StopIteration) is StopIteration:
                    active.remove(item)
                    note_done(h)
                    if todo:
                        active.append(start(todo.pop()))
        for ht in ht_order[(HL - 1) * 8:]:
            at_load(nc.gpsimd, ht)

        # ---- phase D: output projection for this core's 512 rows ----
        # ht order follows AllToAll arrival: all tiles of local head 0
        # first, then head 1, etc., so accumulation overlaps the later
        # collectives.  Remaining at_sb loads are emitted just before the
        # first wo tile that consumes them, on the opposite DMA queue.
        def d_alloc(cg):
            # alternate accumulator pools so cg+1's matmuls start while
            # cg's banks are still draining through the output copies
            if cg % 2 == 0:
                return [
                    ps1.tile([128, 512], F32, tag="p1", name=f"po{cg}_{i}")[:]
                    for i in range(4)
                ]
            pp = [
                ps2.tile([128, 1024], F32, tag="p2", name=f"pp{cg}_{i}")
                for i in range(2)
            ]
            return [pp[0][:, 0:512], pp[0][:, 512:1024],
                    pp[1][:, 0:512], pp[1][:, 512:1024]]

        def d_accum(cg, po, lo, hi):
            for n_ht in range(lo, hi):
                ht = ht_order[n_ht]
                if cg == 0 and ht in pre_wt:
                    wt = pre_wt[ht]
                else:
                    wt = ws.tile([128, 512], BF, tag="wo")
                    weng = (nc.sync, nc.scalar)[n_ht % 2]
                    weng.dma_start(
                        wt[:],
                        woT[ht * 128: (ht + 1) * 128, cg * 512: (cg + 1) * 512],
                    )
                for rt in range(4):
                    nc.tensor.matmul(
                        po[rt],
                        at_sb[:, ht * 512 + rt * 128: ht * 512 + (rt + 1) * 128],
                        wt[:],
                        start=(n_ht == 0), stop=(n_ht == 31),
                    )

        def d_drain(cg, po):
            for rt in range(4):
                ot = osp.tile([128, 512], F32, tag="o")
                nc.vector.tensor_copy(ot[:], po[rt])
                nc.scalar.dma_start(
                    out[rt * 128: (rt + 1) * 128, cg * 512: (cg + 1) * 512], ot[:]
                )

        # cg0/cg1 run as two interleaved passes: cg0's tail (which needs
        # the last AllToAll's tiles) is emitted after cg1's independent
        # head, so the PE queue never head-of-line blocks on collective 3.
        po0, po1 = d_alloc(0), d_alloc(1)
        d_accum(0, po0, 0, 20)
        d_accum(1, po1, 0, 20)
        d_accum(0, po0, 20, 32)
        d_drain(0, po0)
        d_accum(1, po1, 20, 32)
        d_drain(1, po1)
        for cg in range(2, 8):
            po = d_alloc(cg)
            d_accum(cg, po, 0, 32)
            d_drain(cg, po)


def _build():
    nc = bacc.Bacc("TRN2", target_bir_lowering=False, debug=False, num_devices=NC)
    xT = nc.dram_tensor("xT", [D, BS], BF, kind="ExternalInput")
    wqkvT = nc.dram_tensor("wqkvT", [D, 768], BF, kind="ExternalInput")
    woT = nc.dram_tensor("woT", [D, D], BF, kind="ExternalInput")
    ccR = nc.dram_tensor("ccR", [128, NRB * 256], BF, kind="ExternalInput")
    ssR = nc.dram_tensor("ssR", [128, NRB * 256], BF, kind="ExternalInput")
    trim = nc.dram_tensor("trim", [128, 128], F32, kind="ExternalInput")
    ones = nc.dram_tensor("ones", [128, 128], BF, kind="ExternalInput")
    iden = nc.dram_tensor("iden", [128, 128], BF, kind="ExternalInput")
    out = nc.dram_tensor("out", [R, D], F32, kind="ExternalOutput")
    with tile.TileContext(nc) as tc:
        _emit(nc, tc, (xT, wqkvT, woT, ccR, ssR, trim, ones, iden, out))
    nc.compile()
    return nc


_NC = None


def kernel(x, wq, wk, wv, wo, freqs_cos, freqs_sin, mask, start_pos):
    global _NC
    if _NC is None:
        _NC = _build()
    nc = _NC
    bf = ml_dtypes.bfloat16

    x = np.asarray(x, dtype=np.float32)
    xT = np.ascontiguousarray(x.reshape(BS, D).T).astype(bf)

    perm = np.concatenate([np.arange(0, HD, 2), np.arange(1, HD, 2)])
    wqTp = np.asarray(wq, np.float32).T.reshape(D, H, HD)[:, :, perm]
    wkTp = np.asarray(wk, np.float32).T.reshape(D, HKV, HD)[:, :, perm]
    wvT = np.asarray(wv, np.float32).T.reshape(D, HKV, HD)
    woT = np.ascontiguousarray(np.asarray(wo, np.float32).T).astype(bf)

    fc = np.asarray(freqs_cos, np.float32)
    fs = np.asarray(freqs_sin, np.float32)
    # row-major RoPE tables per row block, replicated x4 along free axis
    pos = (np.arange(BS) % S).reshape(NRB, 128)
    ccR = np.tile(fc[pos], (1, 1, 4)).transpose(1, 0, 2).reshape(128, NRB * 256)
    ssR = np.tile(fs[pos], (1, 1, 4)).transpose(1, 0, 2).reshape(128, NRB * 256)
    ccR = np.ascontiguousarray(ccR).astype(bf)
    ssR = np.ascontiguousarray(ssR).astype(bf)

    trim = np.where(
        np.arange(128)[:, None] > np.arange(128)[None, :], -1e30, 0.0
    ).astype(np.float32)
    ones = np.ones((128, 128), dtype=bf)
    iden = np.eye(128, dtype=bf)

    in_maps = []
    for c in range(NC):
        wqkv = np.concatenate(
            [
                wqTp[:, 4 * c: 4 * c + 4].reshape(D, 512),
                wkTp[:, c],
                wvT[:, c],
            ],
            axis=1,
        ).astype(bf)
        in_maps.append(
            {
                "xT": xT,
                "wqkvT": np.ascontiguousarray(wqkv),
                "woT": woT,
                "ccR": ccR,
                "ssR": ssR,
                "trim": trim,
                "ones": ones,
                "iden": iden,
            }
        )

    res = bass_utils.run_bass_kernel_spmd(
        nc, in_maps, core_ids=list(range(NC)), trace=PROFILE, tmpdir=TMPDIR
    )
    if PROFILE:
        print(f"HW exec time: {res.exec_time_ns} ns")
        if res.instructions_and_trace is not None:
            print(f"trace: {res.instructions_and_trace[1]}")

    out_full = np.empty((BS, D), dtype=np.float32)
    for c in range(NC):
        out_full[R * c: R * (c + 1)] = res.results[c]["out"]
    return out_full.reshape(B, S, D)
